# revision 4
# baseline (speedup 1.0000x reference)
import math
import numpy as np

NHEADS = 16
KVHEADS = 4
D = 128
THETA = 10000.0
B, S, E = 2, 2048, 2048
SQ = 512  # s-block width per quarter
NQ = 4

_CACHE = {}


def _build():
    import concourse.bacc as bacc
    from concourse import mybir
    from concourse.tile import TileContext

    f32 = mybir.dt.float32
    f32r = mybir.dt.float32r
    EXP = mybir.ActivationFunctionType.Exp
    COPY = mybir.ActivationFunctionType.Copy

    nc = bacc.Bacc(None, target_bir_lowering=False)
    xT_d = nc.declare_dram_parameter("xT", [E, S], f32r, isOutput=False)
    wq_d = nc.declare_dram_parameter("wq", [E, 512], f32r, isOutput=False)
    wk_d = nc.declare_dram_parameter("wk", [E, 128], f32r, isOutput=False)
    wv_d = nc.declare_dram_parameter("wv", [E, 128], f32r, isOutput=False)
    wd_d = nc.declare_dram_parameter("wd", [512, E], f32r, isOutput=False)
    cos_d = nc.declare_dram_parameter("cosT", [128, S], f32, isOutput=False)
    sin_d = nc.declare_dram_parameter("ssinT", [128, S], f32, isOutput=False)
    msk_d = nc.declare_dram_parameter("masks", [128, 4, 512], f32, isOutput=False)
    ones_d = nc.declare_dram_parameter("ones", [128, 128], f32r, isOutput=False)
    id_d = nc.declare_dram_parameter("ident", [128, 128], f32, isOutput=False)
    out_d = nc.declare_dram_parameter("out", [S, E], f32, isOutput=True)

    xT_r = xT_d.rearrange("(b p) s -> p b s", p=128)  # [128,16,S]
    wq_r = wq_d.rearrange("(b p) c -> p b c", p=128)  # [128,16,512]
    wk_r = wk_d.rearrange("(b p) c -> p b c", p=128)  # [128,16,128]
    wv_r = wv_d.rearrange("(b p) c -> p b c", p=128)
    wd_r = wd_d.rearrange("(h p) e -> p h e", p=128)  # [128,4,E]

    with TileContext(nc) as tc:
        with tc.tile_pool(name="persist", bufs=1) as pp, tc.tile_pool(
            name="xpool", bufs=2
        ) as xpool, tc.tile_pool(name="qpool", bufs=6) as qpool, tc.tile_pool(
            name="scratch", bufs=1
        ) as scr, tc.tile_pool(name="cspool", bufs=2) as cspool, tc.tile_pool(
            name="epool", bufs=3
        ) as epool, tc.tile_pool(name="apool", bufs=6) as apool, tc.tile_pool(
            name="opool", bufs=2
        ) as opool, tc.tile_pool(name="pmm", bufs=4, space="PSUM") as pmm, tc.tile_pool(
            name="pu", bufs=2, space="PSUM"
        ) as pu, tc.tile_pool(name="pn", bufs=2, space="PSUM") as pn:
            # ---- persistent weights / tables ----
            wq_sb = pp.tile([128, 16, 512], f32r, tag="wq")
            for eb in range(16):
                nc.sync.dma_start(out=wq_sb[:, eb, :], in_=wq_r[:, eb, :])
            wk_sb = pp.tile([128, 16, 128], f32r, tag="wk")
            nc.sync.dma_start(out=wk_sb[:], in_=wk_r[:])
            wv_sb = pp.tile([128, 16, 128], f32r, tag="wv")
            nc.sync.dma_start(out=wv_sb[:], in_=wv_r[:])
            msk_sb = pp.tile([128, 4, 512], f32, tag="msk")
            nc.sync.dma_start(out=msk_sb[:], in_=msk_d[:])
            ones_sb = pp.tile([128, 128], f32r, tag="ones")
            nc.sync.dma_start(out=ones_sb[:], in_=ones_d[:])
            id_sb = pp.tile([128, 128], f32, tag="ident")
            nc.sync.dma_start(out=id_sb[:], in_=id_d[:])
            wd_sb = pp.tile([128, 4, 2048], f32r, tag="wd")
            nc.sync.dma_start(out=wd_sb[:], in_=wd_r[:])
            kT_sb = pp.tile([128, 2048], f32r, tag="kT")
            v_sb = pp.tile([128, 16, 128], f32r, tag="v")

            def rope(ps, out_ap, cos_q, sin_q, nm):
                qs = scr.tile([128, 512], f32, tag="qs", bufs=2, name=f"qs{nm}")
                nc.scalar.activation(qs[:], ps[:], COPY)
                qsh = scr.tile([128, 512], f32, tag="qsh", bufs=1, name=f"qsh{nm}")
                nc.vector.tensor_copy(qsh[0:64, :], qs[64:128, :])
                nc.vector.tensor_copy(qsh[64:128, :], qs[0:64, :])
                tc_ = scr.tile([128, 512], f32, tag="tc", bufs=1, name=f"tc{nm}")
                nc.vector.tensor_mul(tc_[:], qs[:], cos_q[:])
                ts_ = scr.tile([128, 512], f32, tag="ts", bufs=1, name=f"ts{nm}")
                nc.vector.tensor_mul(ts_[:], qsh[:], sin_q[:])
                nc.vector.tensor_add(out_ap, tc_[:], ts_[:])

            for sq in range(NQ):
                # ---- stream x quarter (two halves) + rope tables ----
                xh = []
                for half in range(2):
                    t = xpool.tile(
                        [128, 8, 512], f32r, tag="xq", name=f"x{sq}_{half}"
                    )
                    nc.sync.dma_start(
                        out=t[:],
                        in_=xT_r[:, half * 8 : half * 8 + 8, sq * 512 : (sq + 1) * 512],
                    )
                    xh.append(t)
                cos_q = cspool.tile([128, 512], f32, tag="cos", name=f"cos{sq}")
                nc.sync.dma_start(out=cos_q[:], in_=cos_d[:, sq * 512 : (sq + 1) * 512])
                sin_q = cspool.tile([128, 512], f32, tag="sin", name=f"sin{sq}")
                nc.sync.dma_start(out=sin_q[:], in_=sin_d[:, sq * 512 : (sq + 1) * 512])

                # ---- fused QKV projection: j = 4 q-heads, K, V ----
                qt = []
                for j in range(6):
                    ps = pmm.tile([128, 512], f32, tag="mm", name=f"ps{sq}_{j}")
                    for eb in range(16):
                        if j < 4:
                            lhsT = wq_sb[:, eb, j * 128 : (j + 1) * 128]
                        elif j == 4:
                            lhsT = wk_sb[:, eb, :]
                        else:
                            lhsT = wv_sb[:, eb, :]
                        nc.tensor.matmul(
                            ps[:],
                            lhsT,
                            xh[eb // 8][:, eb % 8, :],
                            start=(eb == 0),
                            stop=(eb == 15),
                        )
                    if j < 4:
                        q_t = qpool.tile([128, 512], f32r, tag="qt", name=f"q{sq}_{j}")
                        rope(ps, q_t[:], cos_q, sin_q, f"{sq}_{j}")
                        qt.append(q_t)
                    elif j == 4:
                        rope(
                            ps,
                            kT_sb[:, sq * 512 : (sq + 1) * 512],
                            cos_q,
                            sin_q,
                            f"{sq}_k",
                        )
                    else:
                        vt = scr.tile([128, 512], f32, tag="vt", bufs=2, name=f"vt{sq}")
                        nc.scalar.activation(vt[:], ps[:], COPY)
                        for t4 in range(4):
                            tp = pmm.tile([128, 128], f32, tag="mm", name=f"tp{sq}_{t4}")
                            nc.tensor.transpose(
                                tp[:], vt[:, t4 * 128 : (t4 + 1) * 128], id_sb[:]
                            )
                            nc.scalar.activation(
                                v_sb[:, 4 * sq + t4, :], tp[:], COPY
                            )

                # ---- attention for this q-quarter ----
                nk = 4 * sq + 4
                pend = None  # (u_ps, n_ps, h)

                def flush_pend():
                    nonlocal pend
                    if pend is None:
                        return
                    u_ps, n_ps, hh = pend
                    pend = None
                    recip = scr.tile(
                        [1, 512], f32r, tag="recip", bufs=2, name=f"rc{sq}_{hh}"
                    )
                    with nc.allow_low_precision(reason="f32r is bitwise f32"):
                        nc.vector.reciprocal(recip[:], n_ps[:])
                    rb_ps = pmm.tile([128, 512], f32, tag="mm", name=f"rb{sq}_{hh}")
                    nc.tensor.matmul(
                        rb_ps[:], ones_sb[0:1, 0:128], recip[:], start=True, stop=True
                    )
                    rb = scr.tile(
                        [128, 512], f32, tag="rb", bufs=2, name=f"rbs{sq}_{hh}"
                    )
                    nc.scalar.activation(rb[:], rb_ps[:], COPY)
                    a_t = apool.tile(
                        [128, 512], f32r, tag="a", name=f"a{sq}_{hh}"
                    )
                    nc.vector.tensor_mul(a_t[:], u_ps[:], rb[:])
                    a_tiles[hh] = a_t

                a_tiles = [None] * 4
                for h in range(4):

                    def make_e(kb):
                        s_ps = pmm.tile([128, 512], f32, tag="mm", name=f"s{sq}_{h}_{kb}")
                        nc.tensor.matmul(
                            s_ps[:],
                            kT_sb[:, kb * 128 : (kb + 1) * 128],
                            qt[h][:],
                            start=True,
                            stop=True,
                        )
                        m = kb - 4 * sq
                        if m < 0:
                            e = epool.tile(
                                [128, 512], f32r, tag="e", name=f"e{sq}_{h}_{kb}"
                            )
                            nc.scalar.activation(e[:], s_ps[:], EXP)
                        else:
                            er = epool.tile(
                                [128, 512],
                                f32,
                                tag="eraw",
                                bufs=2,
                                name=f"er{sq}_{h}_{kb}",
                            )
                            nc.scalar.activation(er[:], s_ps[:], EXP)
                            e = epool.tile(
                                [128, 512], f32r, tag="e", name=f"e{sq}_{h}_{kb}"
                            )
                            nc.vector.tensor_mul(e[:], er[:], msk_sb[:, m, :])
                        return e

                    u_ps = pu.tile([128, 512], f32, tag="u", name=f"u{sq}_{h}")
                    n_ps = pn.tile([1, 512], f32, tag="n", name=f"n{sq}_{h}")
                    e_prev = make_e(0)
                    if nk > 1:
                        e_next = make_e(1)
                    flush_pend()  # prior head's normalize, after 2 S^T issued
                    for kb in range(nk):
                        nc.tensor.matmul(
                            u_ps[:],
                            v_sb[:, kb, :],
                            e_prev[:],
                            start=(kb == 0),
                            stop=(kb == nk - 1),
                        )
                        nc.tensor.matmul(
                            n_ps[:],
                            ones_sb[:, 0:1],
                            e_prev[:],
                            start=(kb == 0),
                            stop=(kb == nk - 1),
                        )
                        if kb + 1 < nk:
                            e_prev = e_next
                            e_next = make_e(kb + 2) if kb + 2 < nk else None
                    pend = (u_ps, n_ps, h)
                flush_pend()

                # ---- dense partial: out[q, e] += sum_h,d a[d,q] * wd[d,h,e] ----
                for nb in range(4):
                    for sc in range(4):
                        d_ps = pmm.tile([128, 512], f32, tag="mm", name=f"d{sq}_{nb}_{sc}")
                        for h in range(4):
                            nc.tensor.matmul(
                                d_ps[:],
                                a_tiles[h][:, sc * 128 : (sc + 1) * 128],
                                wd_sb[:, h, nb * 512 : (nb + 1) * 512],
                                start=(h == 0),
                                stop=(h == 3),
                            )
                        o = opool.tile([128, 512], f32, tag="o", name=f"o{sq}_{nb}_{sc}")
                        nc.scalar.activation(o[:], d_ps[:], COPY)
                        nc.sync.dma_start(
                            out=out_d[
                                sq * 512 + sc * 128 : sq * 512 + (sc + 1) * 128,
                                nb * 512 : (nb + 1) * 512,
                            ],
                            in_=o[:],
                        )
    nc.compile()
    return nc


def _host_inputs(x, w_qkv, w_dense):
    scale = 1.0 / math.sqrt(D)
    pos = np.arange(S, dtype=np.float64)
    invf = 1.0 / (THETA ** (np.arange(0, D, 2, dtype=np.float64) / D))
    ang = pos[:, None] * invf[None, :]  # [S, 64]
    cos_h = np.cos(ang)
    sin_h = np.sin(ang)
    cosT = np.ascontiguousarray(
        np.concatenate([cos_h, cos_h], axis=1).T.astype(np.float32)
    )
    ssinT = np.concatenate([sin_h, sin_h], axis=1).T.astype(np.float32)
    ssinT[0:64, :] *= -1.0
    ssinT = np.ascontiguousarray(ssinT)
    kk = np.arange(128)[:, None, None]
    mm = np.arange(4)[None, :, None]
    qq = np.arange(512)[None, None, :]
    masks = ((kk + 128 * mm) <= qq).astype(np.float32)
    ones = np.ones((128, 128), np.float32)
    ident = np.eye(128, dtype=np.float32)
    in_maps = []
    for c in range(8):
        b, g = divmod(c, 4)
        in_maps.append(
            {
                "xT": np.ascontiguousarray(x[b].T),
                "wq": np.ascontiguousarray(w_qkv[:, 512 * g : 512 * (g + 1)])
                * np.float32(scale),
                "wk": np.ascontiguousarray(
                    w_qkv[:, 2048 + 128 * g : 2048 + 128 * (g + 1)]
                ),
                "wv": np.ascontiguousarray(
                    w_qkv[:, 2560 + 128 * g : 2560 + 128 * (g + 1)]
                ),
                "wd": np.ascontiguousarray(w_dense[512 * g : 512 * (g + 1), :]),
                "cosT": cosT,
                "ssinT": ssinT,
                "masks": masks,
                "ones": ones,
                "ident": ident,
            }
        )
    return in_maps


def kernel(x, w_qkv, w_dense):
    import concourse.bass_utils as bass_utils

    if "nc" not in _CACHE:
        _CACHE["nc"] = _build()
    nc = _CACHE["nc"]
    in_maps = _host_inputs(x, w_qkv, w_dense)
    res = bass_utils.run_bass_kernel_spmd(nc, in_maps, list(range(8)), trace=False)
    outs = [res.results[c]["out"] for c in range(8)]
    full = np.empty((B, S, E), np.float32)
    for b in range(B):
        full[b] = outs[4 * b] + outs[4 * b + 1] + outs[4 * b + 2] + outs[4 * b + 3]
    return full
